# revision 7
# baseline (speedup 1.0000x reference)
"""Talking-heads attention Trainium2 kernel, v2 (8-core data-parallel over batch).

Same math as the baseline kernel (see reference.py) but restructured to
minimize instruction count and data movement:

  * AV (attn @ V) runs at free-dim 256 (one chunk-pair) instead of 128:
    384 -> 96 matmuls per chunk-pair... (12h x 8jb at free 256).
  * All DMA transposes are batched: one xbar call per sT half (was 8),
    one per 2 row-groups for pT (was 1 per group), one per x quarter.
  * QK PSUM evictions write 8 key-blocks per DVE scatter (was 2).
  * Weights arrive in one packed dram tensor (single DMA); x in 4 DMAs.
  * Scale folded into Wq host-side; output bias via two ones-matmuls
    per 128-row chunk; y stored fp16 and upcast host-side.

Layouts (per core, B=2 batch items, N=1024, 12 heads x 64):
  xT   [128, 8, 6, 128]   x^T: [dim-in-tile, i-block, k-tile, i-in-block]
  qT/kT[128, 6, 1024]     per-head-column projections (d on partitions)
  v_t  [128, 8, 768]      V with j on partitions
  sT   [128, 2, 8, 16, 128]  S^T per chunk-pair: [j-in-blk, half, jb, ig, (iloc*12+h pad 128)]
  sInt [128, 8, 16, 128]  premix layout: [(iloc*12+h) pad, jb, ig, j-in-blk]
  p2   [96, 2, 1024]      postmixed P'' rows (h*8+iloc) for 2 row-groups
  pT   [128, 32, 8, 96]   P''^T per chunk-pair: [j-in-blk, ig, jb, (h*8+iloc)]
  oaT  [128, 6, 256]      attention out: [(h%2)*64+d, head-pair, i]
"""

import numpy as np
import ml_dtypes

import concourse.bass as bass
import concourse.bacc as bacc
import concourse.mybir as mybir
import concourse.tile as tile
from concourse.bass_utils import run_bass_kernel_spmd
from contextlib import ExitStack

FP16_NP = np.float16

F32 = mybir.dt.float32
FP16 = mybir.dt.float16
F32R = mybir.dt.float32r

B_TOTAL = 16
N_CORES = 8
B = B_TOTAL // N_CORES  # 2 batch items per core
N = 1024
DIM = 768
H = 12
DH = 64
HC = H * DH             # 768
NK = DIM // 128         # 6 contraction tiles
SCALE = DH ** -0.5

ILOC = 8                # query rows per row-group
GRP = ILOC * H          # 96 live partitions per row-group
NJB = N // 128          # 8 key blocks
NCP = N // 256          # 4 chunk-pairs per batch item (256 rows each)
NIG = 32                # row-groups per chunk-pair


def build_program(debug_taps=(), reps=1):
    nc = bacc.Bacc(
        "TRN2",
        target_bir_lowering=False,
        debug=False,
        num_devices=N_CORES,
    )
    taps = {}
    for name, shape, dt in debug_taps:
        taps[name] = nc.declare_dram_parameter(name, list(shape), dt,
                                               isOutput=True)

    x_d = nc.declare_dram_parameter("xb", [B, N, DIM], FP16, isOutput=False)
    w3_d = nc.declare_dram_parameter("w3b", [DIM, 3 * HC], FP16, isOutput=False)
    wo_d = nc.declare_dram_parameter("wob", [HC, DIM], FP16, isOutput=False)
    bo_d = nc.declare_dram_parameter("bob", [1, DIM], FP16, isOutput=False)
    bdpre_d = nc.declare_dram_parameter("bdpre", [GRP, GRP], FP16, isOutput=False)
    bdpat_d = nc.declare_dram_parameter("bdpat", [GRP, GRP], F32, isOutput=False)
    y_d = nc.declare_dram_parameter("y", [B, N, DIM], FP16, isOutput=True)

    with tile.TileContext(nc) as tc:
        with ExitStack() as ctx:
            persist = ctx.enter_context(tc.tile_pool(name="persist", bufs=1))
            work = ctx.enter_context(tc.tile_pool(name="work", bufs=1))
            small = ctx.enter_context(tc.tile_pool(name="small", bufs=2))
            psA = ctx.enter_context(
                tc.tile_pool(name="psA", bufs=1, space="PSUM"))
            psB = ctx.enter_context(
                tc.tile_pool(name="psB", bufs=1, space="PSUM"))

            # ---------------- persistent constants ----------------
            wo_t = persist.tile([128, NK, DIM], FP16, tag="wo")
            nc.sync.dma_start(
                out=wo_t[:],
                in_=wo_d[:, :].rearrange("(k p) c -> p k c", p=128))
            bo_t = persist.tile([1, DIM], FP16, tag="bo")
            nc.sync.dma_start(out=bo_t[:], in_=bo_d[:, :])
            ones_t = persist.tile([1, 128], FP16, tag="ones")
            nc.vector.memset(ones_t[:], 1.0)
            bd_pre = persist.tile([GRP, GRP], FP16, tag="bdpre")
            nc.sync.dma_start(out=bd_pre[:], in_=bdpre_d[:, :])
            bd_pat = persist.tile([GRP, GRP], F32, tag="bdpat")
            nc.sync.dma_start(out=bd_pat[:], in_=bdpat_d[:, :])

            qT = persist.tile([128, NK, N], FP16, tag="qT")
            kT = persist.tile([128, NK, N], FP16, tag="kT")
            v_t = persist.tile([128, NJB, HC], FP16, tag="v")
            # S^T staging; columns 96:128 of each block stay zero forever.
            sT = persist.tile([128, 2, NJB, 16, 128], FP16, tag="sT")
            nc.vector.memset(sT[:], 0.0)

            for rep in range(reps):
             for b in range(B):
                # -------- x load + transpose (4 quarters) --------
                xT = work.tile([128, 8, NK, 128], FP16, tag="sInt")
                for q8 in range(8):
                    x_bf = work.tile([128, DIM], FP16, tag="xbf")
                    nc.sync.dma_start(
                        out=x_bf[:],
                        in_=x_d[b, q8 * 128:(q8 + 1) * 128, :])
                    nc.sync.dma_start_transpose(
                        out=xT[:, q8, :, :], in_=x_bf[:])

                # -------- packed QKV weights (single DMA) --------
                w3 = work.tile([128, NK, 3 * HC], FP16, tag="big")
                nc.sync.dma_start(
                    out=w3[:],
                    in_=w3_d[:, :].rearrange("(k p) c -> p k c", p=128))

                # -------- Q, K projections --------
                for ot in range(12):     # 0..5 -> qT, 6..11 -> kT
                    dst = qT if ot < 6 else kT
                    pp = psA.tile([128, 2, 512], F32, tag="A")
                    for kt in range(NK):
                        for ic in range(2):
                            nc.tensor.matmul(
                                pp[:, ic, :],
                                lhsT=w3[:, kt, ot * 128:(ot + 1) * 128],
                                rhs=xT[:, 4 * ic:4 * ic + 4, kt, :],
                                start=(kt == 0), stop=(kt == NK - 1))
                    nc.vector.tensor_copy(dst[:, ot % 6, :], pp[:])

                # -------- V projection --------
                for jbl in range(NJB):
                    pv = psA.tile([128, 2, 512], F32, tag="A")
                    for kt in range(NK):
                        for nh in range(2):
                            nc.tensor.matmul(
                                pv[:, nh, 0:384],
                                lhsT=xT[:, jbl, kt, :],
                                rhs=w3[:, kt,
                                       2 * HC + nh * 384:2 * HC + nh * 384 + 384],
                                start=(kt == 0), stop=(kt == NK - 1))
                    nc.vector.tensor_copy(
                        v_t[:, jbl, :].rearrange("p (a c) -> p a c", a=2),
                        pv[:, :, 0:384])

                if b == 0 and rep == 0:
                    for nm, src in (("dbg_qT", qT), ("dbg_kT", kT),
                                    ("dbg_v", v_t), ("dbg_xT", xT)):
                        if nm in taps:
                            nc.sync.dma_start(out=taps[nm][:], in_=src[:])

                # ---------------- attention ----------------
                for cp in range(NCP):
                    # QK^T: per head, 8 key-blocks into one 4-bank tile.
                    for h in range(H):
                        ht, hr = divmod(h, 2)
                        qk = psA.tile([128, NJB, 256], F32, tag="A")
                        for jb in range(NJB):
                            nc.tensor.matmul(
                                qk[:, jb, :],
                                lhsT=kT[hr * 64:(hr + 1) * 64, ht,
                                        jb * 128:(jb + 1) * 128],
                                rhs=qT[hr * 64:(hr + 1) * 64, ht,
                                       cp * 256:(cp + 1) * 256],
                                start=(jb % 2 == 0), stop=(jb % 2 == 1))
                        # scatter into sT interleaved columns (4D APs)
                        for hf in range(2):
                            nc.vector.tensor_copy(
                                sT[:, hf, :, :, h:h + GRP:H],
                                qk[:, :, 128 * hf:128 * hf + 128]
                                .rearrange("p j (a c) -> p j a c", c=ILOC))

                    if b == 0 and cp == 0 and rep == 0 and "dbg_sT" in taps:
                        nc.sync.dma_start(out=taps["dbg_sT"][:], in_=sT[:])

                    pT = work.tile([128, NIG, NJB, GRP], FP16, tag="big")
                    for half in range(2):
                        sInt = work.tile([128, NJB, 16, 128], FP16,
                                         tag="sInt")
                        nc.sync.dma_start_transpose(
                            out=sInt[:], in_=sT[:, half, :, :, :])
                        if (b == 0 and cp == 0 and half == 0 and rep == 0
                                and "dbg_sInt" in taps):
                            nc.sync.dma_start(out=taps["dbg_sInt"][:],
                                              in_=sInt[:])

                        for ig in range(16):
                            psm = psB.tile([GRP, N], F32, tag="psm")
                            for jj in range(2):
                                nc.tensor.matmul(
                                    psm[:, jj * 512:(jj + 1) * 512],
                                    lhsT=bd_pre[:],
                                    rhs=sInt[0:GRP, 4 * jj:4 * jj + 4, ig, :],
                                    start=True, stop=True)
                            e_sb = work.tile([GRP, N], F32R, tag="esb")
                            ssum = small.tile([GRP, 1], F32, tag="ssum")
                            nc.scalar.activation(
                                e_sb[:], psm[:],
                                mybir.ActivationFunctionType.Exp,
                                accum_out=ssum[:])
                            recip = small.tile([GRP, 1], F32, tag="recip")
                            nc.vector.reciprocal(recip[:], ssum[:])
                            bd_ps = small.tile([GRP, GRP], F32R, tag="bdps")
                            nc.vector.tensor_scalar_mul(bd_ps[:], bd_pat[:],
                                                        recip[:])
                            if (b == 0 and cp == 0 and half == 0 and ig == 0
                                    and rep == 0 and "dbg_E" in taps):
                                nc.sync.dma_start(out=taps["dbg_E"][:],
                                                  in_=e_sb[:].bitcast(F32))
                            psp = psB.tile([GRP, N], F32, tag="psp")
                            for jj in range(2):
                                nc.tensor.matmul(
                                    psp[:, jj * 512:(jj + 1) * 512],
                                    lhsT=bd_ps[:],
                                    rhs=e_sb[:, jj * 512:(jj + 1) * 512],
                                    start=True, stop=True)
                            if ig % 2 == 0:
                                p2 = work.tile([GRP, 2, N], FP16, tag="p2",
                                               name="p2")
                            nc.vector.tensor_copy(p2[:, ig % 2, :], psp[:])
                            if ig % 2 == 1:
                                igg = half * 16 + ig - 1
                                nc.sync.dma_start_transpose(
                                    out=pT[:, igg:igg + 2, :, :], in_=p2[:])

                    if b == 0 and cp == 0 and rep == 0 and "dbg_pT" in taps:
                        nc.sync.dma_start(out=taps["dbg_pT"][:], in_=pT[:])

                    # -------- attn @ V (free dim 256 = chunk pair) --------
                    oaT = work.tile([128, 6, 256], FP16, tag="oaT")
                    for r in range(3):
                        if r == 0:
                            av = psA.tile([128, 3, 512], F32, tag="A",
                                          name="av")
                        for q2 in range(2):
                            for hh in range(2):
                                h = 4 * r + 2 * q2 + hh
                                for jb in range(NJB):
                                    nc.tensor.matmul(
                                        av[64 * hh:64 * (hh + 1), r,
                                           256 * q2:256 * q2 + 256],
                                        lhsT=v_t[:, jb, h * 64:(h + 1) * 64],
                                        rhs=pT[:, :, jb, 8 * h:8 * h + 8],
                                        start=(q2 == 0 and jb == 0),
                                        stop=(q2 == 1 and jb == NJB - 1),
                                        skip_group_check=True,
                                        tile_position=(0, 64 * hh))
                        nc.vector.tensor_copy(
                            oaT[:, 2 * r:2 * r + 2, :],
                            av[:, r, :].rearrange("p (q c) -> p q c", q=2))

                    if (b == 0 and cp == NCP - 1 and rep == 0
                            and "dbg_oaT" in taps):
                        nc.sync.dma_start(out=taps["dbg_oaT"][:], in_=oaT[:])

                    # -------- output projection + bias --------
                    y_sb = work.tile([128, 2, DIM], FP16, tag="ysb")
                    for c in range(2):
                        psy = psA.tile([128, 2, 512], F32, tag="A")
                        for kt in range(NK):
                            for nh in range(2):
                                nc.tensor.matmul(
                                    psy[:, nh, 0:384],
                                    lhsT=oaT[:, kt, c * 128:(c + 1) * 128],
                                    rhs=wo_t[:, kt, nh * 384:nh * 384 + 384],
                                    start=(kt == 0), stop=False)
                        for nh in range(2):
                            nc.tensor.matmul(
                                psy[:, nh, 0:384], lhsT=ones_t[:],
                                rhs=bo_t[:, nh * 384:nh * 384 + 384],
                                start=False, stop=True)
                        nc.vector.tensor_copy(
                            y_sb[:, c, :].rearrange("p (a c) -> p a c", a=2),
                            psy[:, :, 0:384])
                    nc.sync.dma_start(
                        out=y_d[b, cp * 256:(cp + 1) * 256, :]
                        .rearrange("(c p) d -> p c d", p=128),
                        in_=y_sb[:])

    nc.compile()
    return nc


def host_prep(inputs):
    """Pack weights fp16 (scale folded into Wq) + block mix matrices."""
    mix_pre = np.asarray(inputs["mix_pre"], dtype=np.float32)
    mix_post = np.asarray(inputs["mix_post"], dtype=np.float32)
    bd_pre = np.zeros((GRP, GRP), dtype=np.float32)
    bd_pat = np.zeros((GRP, GRP), dtype=np.float32)
    for i in range(ILOC):
        bd_pre[H * i:H * i + H, H * i:H * i + H] = mix_pre
        for h in range(H):
            bd_pat[H * i:H * i + H, h * ILOC + i] = mix_post[:, h]
    wq = np.asarray(inputs["Wq"], dtype=np.float32) * SCALE
    wkv = np.asarray(inputs["Wkv"], dtype=np.float32)
    w3 = np.concatenate([wq, wkv], axis=1)  # [768, 2304]
    common = {
        "w3b": w3.astype(FP16_NP),
        "wob": np.asarray(inputs["Wo"], dtype=np.float32).astype(FP16_NP),
        "bob": np.asarray(inputs["bo"], dtype=np.float32).reshape(1, DIM)
               .astype(FP16_NP),
        "bdpre": bd_pre.astype(FP16_NP),
        "bdpat": bd_pat,
    }
    return common


def kernel(**inputs):
    x = np.asarray(inputs["x"], dtype=np.float32).astype(FP16_NP)
    common = host_prep(inputs)
    nc = build_program()
    in_maps = []
    for c in range(N_CORES):
        m = dict(common)
        m["xb"] = np.ascontiguousarray(x[c * B:(c + 1) * B])
        in_maps.append(m)
    res = run_bass_kernel_spmd(nc, in_maps, list(range(N_CORES)))
    out = np.concatenate([res.results[c]["y"] for c in range(N_CORES)], axis=0)
    return out.astype(np.float32)


if __name__ == "__main__":
    rng = np.random.default_rng(0)
    ins = {
        "x": rng.standard_normal((B_TOTAL, N, DIM), dtype=np.float32),
        "Wq": rng.standard_normal((DIM, HC), dtype=np.float32) * DIM ** -0.5,
        "Wkv": rng.standard_normal((DIM, 2 * HC), dtype=np.float32) * DIM ** -0.5,
        "mix_pre": rng.standard_normal((H, H), dtype=np.float32),
        "mix_post": rng.standard_normal((H, H), dtype=np.float32),
        "Wo": rng.standard_normal((HC, DIM), dtype=np.float32) * HC ** -0.5,
        "bo": np.zeros(DIM, dtype=np.float32),
    }
    y = kernel(**ins)
    print("kernel output", y.shape, y.dtype, float(np.abs(y).max()))
